# revision 5
# baseline (speedup 1.0000x reference)
"""Multi-head attention (B=2, S=2048, D=1024, H=16) on 8 Trainium2 cores.

Sharding: core = (batch b, head-group g): 2 batches x 4 groups of 4 heads.
Each core computes Q/K/V projections for its 256 model columns, causal
attention for its 4 heads, and a partial output projection through its
256 rows of Wo. Host sums the 4 partials per batch (the "all-reduce").

Device-side layout strategy (per core):
  - Host passes query/key/value pre-transposed: xT [D=1024, S=2048].
  - QT/KT [c=256, s] produced directly with W stationary (full-speed MMs).
  - V [s, c] produced with xT stationary, padded with a ones column per
    head so the attnV matmul also yields the softmax denominator l.
  - Scores computed transposed: ST[k, q] = KT_slice^T-matmul; exp on
    ScalarE with fused 1/sqrt(64) scale (max-subtraction skipped: scores
    are bounded by construction, |s/8| < ~4).
  - attnV: outT[d(+l), q] = V_aug^T @ PT, accumulated over k blocks.
  - Normalize by 1/l via a ones-vector PE broadcast, write OT [c, q].
  - Output projection: lhsT = OT chunks, rhs = Wo -> partial out [s, e].
All matmuls use float32r (full PE speed, fp32 storage).
"""

import os
import numpy as np
from contextlib import ExitStack

import concourse.bass as bass
import concourse.tile as tile
from concourse import bacc, mybir
from concourse import bass_utils
from concourse.bass import ts

B, S, D, H = 2, 2048, 1024, 16
DEPTH = D // H            # 64
NCORES = 8
GROUPS = 4                # head-groups per batch
HG = H // GROUPS          # 4 heads per core
CW = HG * DEPTH           # 256 local columns
P = 128
DC = D // P               # 8 contraction chunks
NST = S // P              # 16 seq tiles of 128
NSB = S // 512            # 4 seq blocks of 512
F32 = mybir.dt.float32
FR = mybir.dt.float32r
SCALE = 1.0 / float(np.sqrt(DEPTH))  # 0.125


def _fr(ap):
    return ap


def _build_program(mode, use_q_bias, use_k_bias, use_v_bias):
    """mode: 'causal' | 'dense' | 'generic'."""
    nc = bacc.Bacc(
        "TRN2",
        target_bir_lowering=False,
        debug=False,
        enable_asserts=False,
        num_devices=NCORES,
    )

    xqT = nc.dram_tensor("xqT", [D, S], FR, kind="ExternalInput").ap()
    xkT = nc.dram_tensor("xkT", [D, S], FR, kind="ExternalInput").ap()
    xvT = nc.dram_tensor("xvT", [D, S], FR, kind="ExternalInput").ap()
    wq = nc.dram_tensor("wq", [D, CW], FR, kind="ExternalInput").ap()
    wk = nc.dram_tensor("wk", [D, CW], FR, kind="ExternalInput").ap()
    wv = nc.dram_tensor("wv", [D, CW], FR, kind="ExternalInput").ap()
    wo = nc.dram_tensor("wo", [CW, D], FR, kind="ExternalInput").ap()
    mtri = None
    mneg = None
    if mode == "causal":
        mtri = nc.dram_tensor("mtri", [P, P], F32, kind="ExternalInput").ap()
    elif mode == "generic":
        mneg = nc.dram_tensor("mneg", [S, S], F32, kind="ExternalInput").ap()
    bq = bk = bv = None
    if use_q_bias:
        bq = nc.dram_tensor("bq", [P, CW // P], F32, kind="ExternalInput").ap()
    if use_k_bias:
        bk = nc.dram_tensor("bk", [P, CW // P], F32, kind="ExternalInput").ap()
    if use_v_bias:
        bv = nc.dram_tensor("bv", [P, CW], F32, kind="ExternalInput").ap()
    out = nc.dram_tensor("out", [S, D], F32, kind="ExternalOutput").ap()

    xqT_r = xqT.rearrange("(dc p) s -> p dc s", p=P)
    xkT_r = xkT.rearrange("(dc p) s -> p dc s", p=P)
    xvT_r = xvT.rearrange("(dc p) s -> p dc s", p=P)
    wq_r = wq.rearrange("(dc p) c -> p dc c", p=P)
    wk_r = wk.rearrange("(dc p) c -> p dc c", p=P)
    wv_r = wv.rearrange("(dc p) c -> p dc c", p=P)
    wo_r = wo.rearrange("(cc p) e -> p cc e", p=P)

    with tile.TileContext(nc) as tc, ExitStack() as ctx:
        wpool = ctx.enter_context(tc.tile_pool(name="wpool", bufs=1))
        xpool = ctx.enter_context(tc.tile_pool(name="xpool", bufs=3))
        qkpool = ctx.enter_context(tc.tile_pool(name="qkpool", bufs=1))
        ptpool = ctx.enter_context(tc.tile_pool(name="ptpool", bufs=4))
        smpool = ctx.enter_context(tc.tile_pool(name="smpool", bufs=2))
        outpool = ctx.enter_context(tc.tile_pool(name="outpool", bufs=3))
        mkpool = ctx.enter_context(tc.tile_pool(name="mkpool", bufs=3))

        # --- persistent SBUF tensors ---
        wq_sb = wpool.tile([P, DC, CW], FR, tag="wq_sb")
        nc.sync.dma_start(wq_sb[:], wq_r)
        wk_sb = wpool.tile([P, DC, CW], FR, tag="wk_sb")
        nc.sync.dma_start(wk_sb[:], wk_r)
        wv_sb = wpool.tile([P, DC, CW], FR, tag="wv_sb")
        nc.sync.dma_start(wv_sb[:], wv_r)
        wo_sb = wpool.tile([P, CW // P, D], FR, tag="wo_sb")
        nc.sync.dma_start(wo_sb[:], wo_r)
        mtri_sb = None
        if mode == "causal":
            mtri_sb = wpool.tile([P, P], F32, tag="mtri_sb")
            nc.sync.dma_start(mtri_sb[:], mtri)
        ones1_f = wpool.tile([1, DEPTH], F32, tag="ones1_f")
        nc.vector.memset(ones1_f[:], 1.0)
        ones1 = wpool.tile([1, DEPTH], FR, tag="ones1")
        nc.vector.tensor_copy(ones1[:], ones1_f[:])
        ones_v = wpool.tile([P, HG, 1], F32, tag="ones_v")
        nc.vector.memset(ones_v[:], 1.0)
        bq_sb = bk_sb = bv_sb = None
        if use_q_bias:
            bq_sb = wpool.tile([P, CW // P], F32, tag="bq_sb")
            nc.sync.dma_start(bq_sb[:], bq)
        if use_k_bias:
            bk_sb = wpool.tile([P, CW // P], F32, tag="bk_sb")
            nc.sync.dma_start(bk_sb[:], bk)
        if use_v_bias:
            bv_sb = wpool.tile([P, CW], F32, tag="bv_sb")
            nc.sync.dma_start(bv_sb[:], bv)

        # Per-slab persistent result tiles (fine-grained deps).
        QT_t = {}  # (cc, sb) -> [128, 512]   c-chunk cc, seq block sb
        KT_t = {}
        OT_t = {}
        for cc in range(CW // P):
            for sb in range(NSB):
                QT_t[(cc, sb)] = qkpool.tile([P, 512], FR, name=f"qt_{cc}_{sb}", tag=f"qt_{cc}_{sb}")
                KT_t[(cc, sb)] = qkpool.tile([P, 512], FR, name=f"kt_{cc}_{sb}", tag=f"kt_{cc}_{sb}")
                OT_t[(cc, sb)] = qkpool.tile([P, 512], FR, name=f"ot_{cc}_{sb}", tag=f"ot_{cc}_{sb}")
        V_t = {}  # st -> [128, HG*(DEPTH+1)]  (ones col per head at 65h+64)
        for st in range(NST):
            V_t[st] = qkpool.tile([P, HG, DEPTH + 1], FR, name=f"v_{st}", tag=f"v_{st}")

        # ---------------- Phase A: V projection ----------------
        with tc.tile_pool(name="pp", bufs=2, space="PSUM") as pp:
            for sl in range(NSB):
                slab = xpool.tile([P, DC, 512], FR, tag="slab")
                nc.sync.dma_start(slab[:], xvT_r[:, :, ts(sl, 512)])
                for sq in range(4):
                    st = sl * 4 + sq
                    psum_v = pp.tile([P, 512], F32, tag="pp")
                    for dc in range(DC):
                        nc.tensor.matmul(
                            psum_v[:, :CW],
                            lhsT=_fr(slab[:, dc, ts(sq, P)]),
                            rhs=_fr(wv_sb[:, dc, :]),
                            start=(dc == 0),
                            stop=(dc == DC - 1),
                        )
                    for h in range(HG):
                        dst = V_t[st][:, h, 0:DEPTH]
                        vsrc = psum_v[:, DEPTH * h : DEPTH * h + DEPTH]
                        if use_v_bias:
                            nc.vector.tensor_tensor(
                                dst, vsrc, bv_sb[:, DEPTH * h : DEPTH * h + DEPTH],
                                mybir.AluOpType.add,
                            )
                        else:
                            nc.vector.tensor_copy(dst, vsrc)
                    nc.vector.tensor_copy(
                        V_t[st][:, :, DEPTH : DEPTH + 1], ones_v[:]
                    )

            # ---------------- Phase B: QT / KT projections ----------------
            for name, x_r, w_sb, b_sb, T_t in (
                ("q", xqT_r, wq_sb, bq_sb, QT_t),
                ("k", xkT_r, wk_sb, bk_sb, KT_t),
            ):
                for sl in range(NSB):
                    slab = xpool.tile([P, DC, 512], FR, tag="slab")
                    nc.sync.dma_start(slab[:], x_r[:, :, ts(sl, 512)])
                    for cc in range(CW // P):
                        psum_q = pp.tile([P, 512], F32, tag="pp")
                        for dc in range(DC):
                            nc.tensor.matmul(
                                psum_q[:],
                                lhsT=_fr(w_sb[:, dc, ts(cc, P)]),
                                rhs=_fr(slab[:, dc, :]),
                                start=(dc == 0),
                                stop=(dc == DC - 1),
                            )
                        if b_sb is not None:
                            nc.vector.tensor_scalar_add(
                                T_t[(cc, sl)][:], psum_q[:], b_sb[:, cc : cc + 1]
                            )
                        else:
                            nc.vector.tensor_copy(T_t[(cc, sl)][:], psum_q[:])

        # ---------------- Phase C: attention + Phase D: output proj ------
        with (
            tc.tile_pool(name="ps", bufs=3, space="PSUM") as ps,
            tc.tile_pool(name="po", bufs=2, space="PSUM") as po,
            tc.tile_pool(name="pb", bufs=1, space="PSUM") as pb,
            tc.tile_pool(name="pf", bufs=2, space="PSUM") as pf,
        ):
            for i in range(NSB):  # query block of 512
                for h in range(HG):
                    cc = h // 2
                    off = DEPTH * (h % 2)
                    jmax = 4 * i + 4 if mode == "causal" else NST
                    psum_o = po.tile([DEPTH + 1, 512], F32, tag="po")
                    for j in range(jmax):
                        psum_s = ps.tile([P, 512], F32, tag="ps")
                        nc.tensor.matmul(
                            psum_s[:],
                            lhsT=_fr(KT_t[(cc, j // 4)][off : off + DEPTH, ts(j % 4, P)]),
                            rhs=_fr(QT_t[(cc, i)][off : off + DEPTH, :]),
                            start=True,
                            stop=True,
                        )
                        if mode == "generic":
                            mk = mkpool.tile([P, 512], F32, tag="mk")
                            nc.sync.dma_start(mk[:], mneg[ts(j, P), ts(i, 512)])
                            nc.vector.tensor_tensor(
                                psum_s[:], psum_s[:], mk[:], mybir.AluOpType.add
                            )
                        pt = ptpool.tile([P, 512], FR, tag="pt")
                        r = j - 4 * i
                        if mode == "causal" and r >= 0:
                            # diagonal-containing tile: cols < 128r fully
                            # masked (skipped entirely), [128r, 128r+128)
                            # gets the additive triangular mask pre-exp
                            lo = P * r
                            nc.vector.tensor_tensor(
                                psum_s[:, lo : lo + P],
                                psum_s[:, lo : lo + P],
                                mtri_sb[:],
                                mybir.AluOpType.add,
                            )
                        else:
                            lo = 0
                        nc.scalar.activation(
                            pt[:, lo:],
                            psum_s[:, lo:],
                            mybir.ActivationFunctionType.Exp,
                            scale=SCALE,
                        )
                        nc.tensor.matmul(
                            psum_o[:, lo:],
                            lhsT=_fr(V_t[j][:, h, :]),
                            rhs=_fr(pt[:, lo:]),
                            start=(j == 0),
                            stop=(j == jmax - 1),
                        )
                    # normalize: rows 0..63 / row 64, write OT
                    recip = smpool.tile([1, 512], FR, tag="recip")
                    with nc.allow_low_precision(
                        reason="f32r storage of softmax denominators (22-bit)"
                    ):
                        nc.vector.reciprocal(recip[:], psum_o[DEPTH : DEPTH + 1, :])
                    psum_bb = pb.tile([DEPTH, 512], F32, tag="pb")
                    nc.tensor.matmul(
                        psum_bb[:], lhsT=_fr(ones1[:]), rhs=_fr(recip[:]),
                        start=True, stop=True,
                    )
                    rb = smpool.tile([DEPTH, 512], F32, tag="rb")
                    nc.vector.tensor_copy(rb[:], psum_bb[:])
                    nc.vector.tensor_tensor(
                        OT_t[(cc, i)][off : off + DEPTH, :],
                        psum_o[:DEPTH, :],
                        rb[:],
                        mybir.AluOpType.mult,
                    )
                # Phase D for this query block (all 4 heads done)
                for qq in range(4):
                    qt = 4 * i + qq
                    for eh in range(2):
                        psum_f = pf.tile([P, 512], F32, tag="pf")
                        for cc2 in range(CW // P):
                            nc.tensor.matmul(
                                psum_f[:],
                                lhsT=_fr(OT_t[(cc2, i)][:, ts(qq, P)]),
                                rhs=_fr(wo_sb[:, cc2, ts(eh, 512)]),
                                start=(cc2 == 0),
                                stop=(cc2 == CW // P - 1),
                            )
                        out_t = outpool.tile([P, 512], F32, tag="out_t")
                        nc.vector.tensor_copy(out_t[:], psum_f[:])
                        nc.sync.dma_start(out[ts(qt, P), ts(eh, 512)], out_t[:])

    nc.compile()
    return nc


_PROG_CACHE = {}


def _get_program(mode, use_q_bias, use_k_bias, use_v_bias):
    key = (mode, use_q_bias, use_k_bias, use_v_bias)
    if key not in _PROG_CACHE:
        _PROG_CACHE[key] = _build_program(mode, use_q_bias, use_k_bias, use_v_bias)
    return _PROG_CACHE[key]


def kernel(**inputs):
    query = np.asarray(inputs["query"], np.float32)
    key = np.asarray(inputs["key"], np.float32)
    value = np.asarray(inputs["value"], np.float32)
    mask = np.asarray(inputs["mask"], np.float32).reshape(S, S)
    wq = np.asarray(inputs["wq"], np.float32)
    wk = np.asarray(inputs["wk"], np.float32)
    wv = np.asarray(inputs["wv"], np.float32)
    wo = np.asarray(inputs["wo"], np.float32)
    bq = np.asarray(inputs["bq"], np.float32)
    bk = np.asarray(inputs["bk"], np.float32)
    bv = np.asarray(inputs["bv"], np.float32)
    bo = np.asarray(inputs["bo"], np.float32)

    if not mask.any():
        mode = "dense"
    elif np.array_equal(mask, np.triu(np.ones((S, S), np.float32), 1)):
        mode = "causal"
    else:
        mode = "generic"
    use_q_bias = bool(bq.any())
    use_k_bias = bool(bk.any())
    use_v_bias = bool(bv.any())

    nc = _get_program(mode, use_q_bias, use_k_bias, use_v_bias)

    in_maps = []
    for core in range(NCORES):
        b, g = core // GROUPS, core % GROUPS
        cs = slice(g * CW, (g + 1) * CW)
        m = {
            "xqT": np.ascontiguousarray(query[b].T),
            "xkT": np.ascontiguousarray(key[b].T),
            "xvT": np.ascontiguousarray(value[b].T),
            "wq": np.ascontiguousarray(wq[:, cs]),
            "wk": np.ascontiguousarray(wk[:, cs]),
            "wv": np.ascontiguousarray(wv[:, cs]),
            "wo": np.ascontiguousarray(wo[cs, :]),
        }
        if mode == "causal":
            m["mtri"] = np.where(
                np.triu(np.ones((P, P), bool), 0), np.float32(0), np.float32(-1e9 / SCALE)
            ).astype(np.float32)
        elif mode == "generic":
            m["mneg"] = np.ascontiguousarray(mask.T) * np.float32(-1e9 / SCALE)
        if use_q_bias:
            m["bq"] = np.ascontiguousarray(bq[cs].reshape(CW // P, P).T)
        if use_k_bias:
            m["bk"] = np.ascontiguousarray(bk[cs].reshape(CW // P, P).T)
        if use_v_bias:
            m["bv"] = np.ascontiguousarray(np.tile(bv[cs], (P, 1)))
        in_maps.append(m)

    res = bass_utils.run_bass_kernel_spmd(
        nc, in_maps, core_ids=list(range(NCORES)), trace=False
    )
    outs = [r["out"] for r in res.results]
    full = np.empty((B, S, D), np.float32)
    for b in range(B):
        full[b] = outs[GROUPS * b]
        for g in range(1, GROUPS):
            full[b] += outs[GROUPS * b + g]
        full[b] += bo
    return full


# revision 8
# speedup vs baseline: 1.4510x; 1.4510x over previous
"""Multi-head attention (B=2, S=2048, D=1024, H=16) on 8 Trainium2 cores.

Sharding: core = (batch b, head-group g): 2 batches x 4 groups of 4 heads.
Each core computes Q/K/V projections for its 256 model columns, causal
attention for its 4 heads, and a partial output projection through its
256 rows of Wo. Host sums the 4 partials per batch (the "all-reduce").

Device-side layout strategy (per core):
  - Host passes query/key/value pre-tiled+transposed: [NSB, 128, 8, 512]
    (contiguous 16KB DMA runs per partition).
  - QT/KT [c=256, s] produced directly with W stationary (full-speed MMs).
  - V [s, c] produced with xT stationary, padded with a ones column per
    head so the attnV matmul also yields the softmax denominator l.
  - Scores computed transposed: ST[k, q], one psum tile per (head-pair, j)
    holding both heads (row-disjoint matmuls overlap in the PE array);
    additive causal mask on the diagonal 128-blocks; exp on ScalarE with
    fused 1/sqrt(64) scale (max-subtraction skipped: scores bounded).
  - attnV: outT[d(+l), q] = V_aug^T @ PT, accumulated over k blocks in
    PSUM; columns below the causal diagonal are skipped entirely.
  - Normalize with reciprocal_approx_fast + GpSimd partition_broadcast
    (PE never stalls on the softmax denominator).
  - Output projection: lhsT = OT chunks, rhs = Wo -> partial out [s, e].
All matmuls use float32r (full PE speed, fp32 storage).
Work is emitted interleaved per 512-seq-block so DMA, PE, ACT, DVE and
GpSimd overlap across phases.
"""

import os
import numpy as np
from contextlib import ExitStack

import concourse.bass as bass
import concourse.tile as tile
from concourse import bacc, mybir
from concourse import bass_utils
from concourse.bass import ts

B, S, D, H = 2, 2048, 1024, 16
DEPTH = D // H            # 64
NCORES = 8
GROUPS = 4                # head-groups per batch
HG = H // GROUPS          # 4 heads per core
CW = HG * DEPTH           # 256 local columns
P = 128
DC = D // P               # 8 contraction chunks
NST = S // P              # 16 seq tiles of 128
NSB = S // 512            # 4 seq blocks of 512
F32 = mybir.dt.float32
FR = mybir.dt.float32r
SCALE = 1.0 / float(np.sqrt(DEPTH))  # 0.125
NEG = np.float32(-1e9 / SCALE)


def _build_program(mode, use_q_bias, use_k_bias, use_v_bias):
    """mode: 'causal' | 'dense' | 'generic'."""
    nc = bacc.Bacc(
        "TRN2",
        target_bir_lowering=False,
        debug=False,
        enable_asserts=False,
        num_devices=NCORES,
    )

    xq = nc.dram_tensor("xq", [NSB, P, DC, 512], FR, kind="ExternalInput").ap()
    xk = nc.dram_tensor("xk", [NSB, P, DC, 512], FR, kind="ExternalInput").ap()
    xv = nc.dram_tensor("xv", [NSB, P, DC, 512], FR, kind="ExternalInput").ap()
    wq = nc.dram_tensor("wq", [P, DC, CW], FR, kind="ExternalInput").ap()
    wk = nc.dram_tensor("wk", [P, DC, CW], FR, kind="ExternalInput").ap()
    wv = nc.dram_tensor("wv", [P, DC, CW], FR, kind="ExternalInput").ap()
    wo = nc.dram_tensor("wo", [P, CW // P, D], FR, kind="ExternalInput").ap()
    mtri = None
    mneg = None
    if mode == "causal":
        mtri = nc.dram_tensor("mtri", [P, P], F32, kind="ExternalInput").ap()
    elif mode == "generic":
        mneg = nc.dram_tensor("mneg", [S, S], F32, kind="ExternalInput").ap()
    bq = bk = bv = None
    if use_q_bias:
        bq = nc.dram_tensor("bq", [P, CW // P], F32, kind="ExternalInput").ap()
    if use_k_bias:
        bk = nc.dram_tensor("bk", [P, CW // P], F32, kind="ExternalInput").ap()
    if use_v_bias:
        bv = nc.dram_tensor("bv", [P, CW], F32, kind="ExternalInput").ap()
    out = nc.dram_tensor("out", [S, D], F32, kind="ExternalOutput").ap()

    with tile.TileContext(nc) as tc, ExitStack() as ctx:
        wpool = ctx.enter_context(tc.tile_pool(name="wpool", bufs=1))
        xpool = ctx.enter_context(tc.tile_pool(name="xpool", bufs=3))
        qkpool = ctx.enter_context(tc.tile_pool(name="qkpool", bufs=1))
        ptpool = ctx.enter_context(tc.tile_pool(name="ptpool", bufs=3))
        smpool = ctx.enter_context(tc.tile_pool(name="smpool", bufs=2))
        outpool = ctx.enter_context(tc.tile_pool(name="outpool", bufs=3))
        mkpool = ctx.enter_context(tc.tile_pool(name="mkpool", bufs=3))
        # PSUM: pf (proj + final, 2x1 bank) + ps (scores pairs, 2x2 banks)
        # + po (attnV accum, 2x1 bank) = 8 banks exactly
        pf = ctx.enter_context(tc.tile_pool(name="pf", bufs=2, space="PSUM"))
        ps = ctx.enter_context(tc.tile_pool(name="ps", bufs=2, space="PSUM"))
        po = ctx.enter_context(tc.tile_pool(name="po", bufs=2, space="PSUM"))

        # --- persistent SBUF tensors ---
        wq_sb = wpool.tile([P, DC, CW], FR, tag="wq_sb")
        nc.sync.dma_start(wq_sb[:], wq)
        wk_sb = wpool.tile([P, DC, CW], FR, tag="wk_sb")
        nc.sync.dma_start(wk_sb[:], wk)
        wv_sb = wpool.tile([P, DC, CW], FR, tag="wv_sb")
        nc.sync.dma_start(wv_sb[:], wv)
        wo_sb = wpool.tile([P, CW // P, D], FR, tag="wo_sb")
        nc.sync.dma_start(wo_sb[:], wo)
        mtri_sb = None
        if mode == "causal":
            mtri_sb = wpool.tile([P, P], F32, tag="mtri_sb")
            nc.sync.dma_start(mtri_sb[:], mtri)
        ones_v = wpool.tile([P, HG, 1], F32, tag="ones_v")
        nc.vector.memset(ones_v[:], 1.0)
        bq_sb = bk_sb = bv_sb = None
        if use_q_bias:
            bq_sb = wpool.tile([P, CW // P], F32, tag="bq_sb")
            nc.sync.dma_start(bq_sb[:], bq)
        if use_k_bias:
            bk_sb = wpool.tile([P, CW // P], F32, tag="bk_sb")
            nc.sync.dma_start(bk_sb[:], bk)
        if use_v_bias:
            bv_sb = wpool.tile([P, CW], F32, tag="bv_sb")
            nc.sync.dma_start(bv_sb[:], bv)

        # Persistent per-block result tiles (fine-grained deps).
        QT_t = {}  # (cc, sb) -> [128, 512]
        KT_t = {}
        OT_t = {}
        for cc in range(CW // P):
            for sb in range(NSB):
                QT_t[(cc, sb)] = qkpool.tile(
                    [P, 512], FR, name=f"qt_{cc}_{sb}", tag=f"qt_{cc}_{sb}")
                KT_t[(cc, sb)] = qkpool.tile(
                    [P, 512], FR, name=f"kt_{cc}_{sb}", tag=f"kt_{cc}_{sb}")
                OT_t[(cc, sb)] = qkpool.tile(
                    [P, 512], FR, name=f"ot_{cc}_{sb}", tag=f"ot_{cc}_{sb}")
        V_t = {}  # st -> [128, HG, DEPTH+1] (ones col per head)
        for st in range(NST):
            V_t[st] = qkpool.tile(
                [P, HG, DEPTH + 1], FR, name=f"v_{st}", tag=f"v_{st}")

        def project_block(sl):
            # V projection for seq block sl
            slab = xpool.tile([P, DC, 512], FR, tag="slab", name=f"slv_{sl}")
            nc.sync.dma_start(slab[:], xv[sl])
            for sq in range(4):
                st = sl * 4 + sq
                psum_v = pf.tile([P, 512], F32, tag="pf", name=f"pv_{st}")
                for dc in range(DC):
                    nc.tensor.matmul(
                        psum_v[:, :CW],
                        lhsT=slab[:, dc, ts(sq, P)],
                        rhs=wv_sb[:, dc, :],
                        start=(dc == 0),
                        stop=(dc == DC - 1),
                    )
                psrc = psum_v[:, :CW].rearrange("p (h d) -> p h d", h=HG)
                if use_v_bias:
                    nc.vector.tensor_tensor(
                        V_t[st][:, :, 0:DEPTH], psrc,
                        bv_sb.rearrange("p (h d) -> p h d", h=HG),
                        mybir.AluOpType.add,
                    )
                else:
                    nc.vector.tensor_copy(V_t[st][:, :, 0:DEPTH], psrc)
                nc.vector.tensor_copy(
                    V_t[st][:, :, DEPTH : DEPTH + 1], ones_v[:])
            # QT / KT projections for seq block sl
            for nm, x_p, w_sb, b_sb, T_t in (
                ("q", xq, wq_sb, bq_sb, QT_t),
                ("k", xk, wk_sb, bk_sb, KT_t),
            ):
                slab = xpool.tile([P, DC, 512], FR, tag="slab",
                                  name=f"sl{nm}_{sl}")
                nc.sync.dma_start(slab[:], x_p[sl])
                for cc in range(CW // P):
                    psum_q = pf.tile([P, 512], F32, tag="pf",
                                     name=f"p{nm}_{cc}_{sl}")
                    for dc in range(DC):
                        nc.tensor.matmul(
                            psum_q[:],
                            lhsT=w_sb[:, dc, ts(cc, P)],
                            rhs=slab[:, dc, :],
                            start=(dc == 0),
                            stop=(dc == DC - 1),
                        )
                    if b_sb is not None:
                        nc.vector.tensor_scalar_add(
                            T_t[(cc, sl)][:], psum_q[:], b_sb[:, cc : cc + 1])
                    else:
                        nc.vector.tensor_copy(T_t[(cc, sl)][:], psum_q[:])

        def attention_block(i):
            jmax = 4 * i + 4 if mode == "causal" else NST
            for cc in range(CW // P):  # head pair (2cc, 2cc+1)
                po0 = po.tile([DEPTH + 1, 512], F32, tag="po",
                              name=f"po0_{i}_{cc}")
                po1 = po.tile([DEPTH + 1, 512], F32, tag="po",
                              name=f"po1_{i}_{cc}")
                pos = (po0, po1)
                for j in range(jmax):
                    psj = ps.tile([P, 2, 512], F32, tag="ps",
                                  name=f"ps_{i}_{cc}_{j}")
                    for hh in range(2):
                        nc.tensor.matmul(
                            psj[:, hh, :],
                            lhsT=KT_t[(cc, j // 4)][
                                DEPTH * hh : DEPTH * hh + DEPTH, ts(j % 4, P)],
                            rhs=QT_t[(cc, i)][DEPTH * hh : DEPTH * hh + DEPTH, :],
                            start=True,
                            stop=True,
                        )
                    r = j - 4 * i
                    lo = 0
                    if mode == "causal" and r >= 0:
                        lo = P * r
                        nc.vector.tensor_tensor(
                            psj[:, :, lo : lo + P],
                            psj[:, :, lo : lo + P],
                            mtri_sb[:, None, :].to_broadcast((P, 2, P)),
                            mybir.AluOpType.add,
                        )
                    elif mode == "generic":
                        mk = mkpool.tile([P, 512], F32, tag="mk",
                                         name=f"mk_{i}_{cc}_{j}")
                        nc.sync.dma_start(mk[:], mneg[ts(j, P), ts(i, 512)])
                        nc.vector.tensor_tensor(
                            psj[:], psj[:],
                            mk[:, None, :].to_broadcast((P, 2, 512)),
                            mybir.AluOpType.add,
                        )
                    pt = ptpool.tile([P, 2, 512], FR, tag="pt",
                                     name=f"pt_{i}_{cc}_{j}")
                    nc.scalar.activation(
                        pt[:, :, lo:],
                        psj[:, :, lo:],
                        mybir.ActivationFunctionType.Exp,
                        scale=SCALE,
                    )
                    for hh in range(2):
                        nc.tensor.matmul(
                            pos[hh][:, lo:],
                            lhsT=V_t[j][:, 2 * cc + hh, :],
                            rhs=pt[:, hh, lo:],
                            start=(j == 0),
                            stop=(j == jmax - 1),
                        )
                # normalize both heads: OT[c, q] = outT[c, q] / l[q]
                for hh in range(2):
                    l_sb = smpool.tile([1, 512], F32, tag="l_sb",
                                       name=f"l_{i}_{cc}_{hh}")
                    # custom-DVE ops read garbage from PSUM on HW; stage the
                    # denominator row through SBUF first
                    nc.vector.tensor_copy(
                        l_sb[:], pos[hh][DEPTH : DEPTH + 1, :])
                    rl_sb = smpool.tile([1, 512], F32, tag="rl_sb",
                                        name=f"rl_{i}_{cc}_{hh}")
                    nc.vector.reciprocal_approx_fast(
                        out=rl_sb[:], in_=l_sb[:])
                    rb = smpool.tile([DEPTH, 512], F32, tag="rb",
                                     name=f"rb_{i}_{cc}_{hh}")
                    nc.gpsimd.partition_broadcast(rb[:], rl_sb[:])
                    nc.vector.tensor_tensor(
                        OT_t[(cc, i)][DEPTH * hh : DEPTH * hh + DEPTH, :],
                        pos[hh][0:DEPTH, :],
                        rb[:],
                        mybir.AluOpType.mult,
                    )

        def output_block(i):
            for qq in range(4):
                qt = 4 * i + qq
                for eh in range(2):
                    psum_f = pf.tile([P, 512], F32, tag="pf",
                                     name=f"pfin_{qt}_{eh}")
                    for cc2 in range(CW // P):
                        nc.tensor.matmul(
                            psum_f[:],
                            lhsT=OT_t[(cc2, i)][:, ts(qq, P)],
                            rhs=wo_sb[:, cc2, ts(eh, 512)],
                            start=(cc2 == 0),
                            stop=(cc2 == CW // P - 1),
                        )
                    out_t = outpool.tile([P, 512], F32, tag="out_t",
                                         name=f"ot_{qt}_{eh}")
                    nc.vector.tensor_copy(out_t[:], psum_f[:])
                    nc.sync.dma_start(out[ts(qt, P), ts(eh, 512)], out_t[:])

        for sl in range(NSB):
            project_block(sl)
            if mode == "causal":
                attention_block(sl)
                output_block(sl)
        if mode != "causal":
            # dense/generic need all KT/V before any attention block
            for i in range(NSB):
                attention_block(i)
                output_block(i)

    nc.compile()
    return nc


_PROG_CACHE = {}


def _get_program(mode, use_q_bias, use_k_bias, use_v_bias):
    key = (mode, use_q_bias, use_k_bias, use_v_bias)
    if key not in _PROG_CACHE:
        _PROG_CACHE[key] = _build_program(mode, use_q_bias, use_k_bias, use_v_bias)
    return _PROG_CACHE[key]


def _pretile(x2d):
    # [S, D] -> [NSB, P, DC, 512]: arr[sl, p, dc, s] = x2d[sl*512+s, dc*128+p]
    return np.ascontiguousarray(
        x2d.reshape(NSB, 512, DC, P).transpose(0, 3, 2, 1))


def _pretile_w(w):
    # [D, CW] -> [P, DC, CW]
    return np.ascontiguousarray(w.reshape(DC, P, CW).transpose(1, 0, 2))


def kernel(**inputs):
    query = np.asarray(inputs["query"], np.float32)
    key = np.asarray(inputs["key"], np.float32)
    value = np.asarray(inputs["value"], np.float32)
    mask = np.asarray(inputs["mask"], np.float32).reshape(S, S)
    wq = np.asarray(inputs["wq"], np.float32)
    wk = np.asarray(inputs["wk"], np.float32)
    wv = np.asarray(inputs["wv"], np.float32)
    wo = np.asarray(inputs["wo"], np.float32)
    bq = np.asarray(inputs["bq"], np.float32)
    bk = np.asarray(inputs["bk"], np.float32)
    bv = np.asarray(inputs["bv"], np.float32)
    bo = np.asarray(inputs["bo"], np.float32)

    if not mask.any():
        mode = "dense"
    elif np.array_equal(mask, np.triu(np.ones((S, S), np.float32), 1)):
        mode = "causal"
    else:
        mode = "generic"
    use_q_bias = bool(bq.any())
    use_k_bias = bool(bk.any())
    use_v_bias = bool(bv.any())

    nc = _get_program(mode, use_q_bias, use_k_bias, use_v_bias)

    in_maps = []
    for core in range(NCORES):
        b, g = core // GROUPS, core % GROUPS
        cs = slice(g * CW, (g + 1) * CW)
        m = {
            "xq": _pretile(query[b]),
            "xk": _pretile(key[b]),
            "xv": _pretile(value[b]),
            "wq": _pretile_w(wq[:, cs]),
            "wk": _pretile_w(wk[:, cs]),
            "wv": _pretile_w(wv[:, cs]),
            "wo": np.ascontiguousarray(
                wo[cs, :].reshape(CW // P, P, D).transpose(1, 0, 2)),
        }
        if mode == "causal":
            m["mtri"] = np.where(
                np.triu(np.ones((P, P), bool), 0), np.float32(0), NEG
            ).astype(np.float32)
        elif mode == "generic":
            m["mneg"] = np.ascontiguousarray(mask.T) * NEG
        if use_q_bias:
            m["bq"] = np.ascontiguousarray(bq[cs].reshape(CW // P, P).T)
        if use_k_bias:
            m["bk"] = np.ascontiguousarray(bk[cs].reshape(CW // P, P).T)
        if use_v_bias:
            m["bv"] = np.ascontiguousarray(np.tile(bv[cs], (P, 1)))
        in_maps.append(m)

    res = bass_utils.run_bass_kernel_spmd(
        nc, in_maps, core_ids=list(range(NCORES)), trace=False
    )
    outs = [r["out"] for r in res.results]
    full = np.empty((B, S, D), np.float32)
    for b in range(B):
        full[b] = outs[GROUPS * b]
        for g in range(1, GROUPS):
            full[b] += outs[GROUPS * b + g]
        full[b] += bo
    return full
